# revision 45
# baseline (speedup 1.0000x reference)
"""GTLayer (graph transformer layer) distributed Bass kernel for 8 TRN2 cores.

Sharding: nodes (and their incoming edges) are partitioned across 8 cores by
node id (5000 dst nodes/core).  Host-side prep redistributes RAW input rows
per edge (the halo-exchange analog): for each core's dst-sorted, padded edge
list we build transposed per-edge arrays kT[e]=k[src_e], vT, qeT(dst), efT.

Device pipeline (per core), per group of 128 dst nodes (17 edge tiles):
 - 5 batched DMAs bring the group's edge streams into SBUF.
 - Per 512-edge chunk: weight-stationary matmuls produce channel-major
   kp_cm/qe_cm [C, 512] in PSUM; ACT copies kp->SBUF bf16; DVE forms
   prod_cm = qe*kp (bf16).
 - Per 128-edge tile: PE contracts prod_cm slice against a constant
   head-selector [C,H] and accumulates the edge-bias matmul into the same
   PSUM scores tile; ACT exps 4 tiles of scores at once; DVE weights vp by
   attn; PE aggregates (one-hot matmul) values+denominators per group.
 - GpSimd builds the one-hot tiles (is_equal vs iota) off the critical path.
Then Wo + residual + BN (global stats via AllReduce) + FFN + BN, identical
to the reference semantics.
"""

import json
from contextlib import ExitStack
import numpy as np
import ml_dtypes

import concourse.bass as bass
import concourse.mybir as mybir
import concourse.tile as tile
from concourse.bass_utils import run_bass_kernel_spmd

bf16 = ml_dtypes.bfloat16

# problem constants (hardcoded per contract)
N, E, IN, H, D, ED = 40000, 640000, 128, 8, 16, 64
C = H * D            # 128
NCORE = 8
NSH = N // NCORE     # 5000 nodes per core
NG = 40              # node groups of <=128 per core (39*128+8)
SG_TILES = 17        # padded edge tiles per group (17*128 = 2176 slots)
SG = SG_TILES * 128
S = NG * SG          # slots per core
EPS = 1e-5
# chunk structure within a group: 4 chunks of 4 tiles + 1 odd tile
CHUNKS = [4, 4, 4, 4, 1]
assert sum(CHUNKS) == SG_TILES

f32 = mybir.dt.float32
bft = mybir.dt.bfloat16


def _split_multiwaits_json(bir: bytes) -> bytes:
    """This walrus build allows only ONE sem wait per instruction; Tile emits
    multi-waits.  Split extras onto NoOps inserted before, same engine."""
    b = json.loads(bir)
    ctr = [0]
    changed = False
    for f in b.get("functions", []):
        for blk in f.get("blocks", []):
            insts = blk.get("instructions")
            if not insts:
                continue
            out = []
            for i in insts:
                si = i.get("sync_info")
                waits = (si or {}).get("on_wait") or []
                if len(waits) > 1:
                    changed = True
                    for w in waits[:-1]:
                        ctr[0] += 1
                        out.append({
                            "debug": i.get("debug", 0), "engine": i["engine"],
                            "ins": [], "name": f"I-wsplit-{ctr[0]}",
                            "opcode": "NoOp", "outs": [],
                            "text_hint": "wsplit",
                            "sync_info": {"on_update": [], "on_wait": [w]},
                        })
                    si["on_wait"] = [waits[-1]]
                out.append(i)
            blk["instructions"] = out
    return json.dumps(b).encode() if changed else bir


class _BassW(bass.Bass):
    def to_json_bytes(self) -> bytes:
        return _split_multiwaits_json(super().to_json_bytes())


def _build_program():
    nc = _BassW()
    dt_in = {
        "kT": (mybir.dt.float8e4, [IN, S]), "vT": (bft, [IN, S]),
        "qeT": (mybir.dt.float8e4, [IN, S]),
        "efT": (bft, [ED + 1, S]),
        "ohT": (mybir.dt.float8e4, [128, S]),
        "iota": (f32, [128, 128]),
        "sel": (bft, [C, H]),
        "qT": (f32, [IN, NSH]),
        "WkT": (bft, [IN, C]), "WvT": (bft, [IN, C]), "WqT": (bft, [IN, C]),
        "WeT": (bft, [ED + 1, H]),
        "WoT": (bft, [C, C]),
        "W1Ta": (bft, [C, C]), "W1Tb": (bft, [C, C]),
        "W2Ta": (bft, [C, C]), "W2Tb": (bft, [C, C]),
        "b1a": (f32, [128, 1]), "b1b": (f32, [128, 1]), "b2": (f32, [128, 1]),
        "g1": (f32, [128, 1]), "bt1": (f32, [128, 1]),
        "g2": (f32, [128, 1]), "bt2": (f32, [128, 1]),
    }
    dins = {k: nc.dram_tensor(k, sh, dt, kind="ExternalInput")
            for k, (dt, sh) in dt_in.items()}
    dout = nc.dram_tensor("out", [C, NSH], f32, kind="ExternalOutput")

    CH = 500  # phase-2 node chunk
    NCH = NSH // CH

    with tile.TileContext(nc) as tc:
        with (
            tc.tile_pool(name="const", bufs=1) as cpool,
            tc.tile_pool(name="wts", bufs=1) as wpool,
            tc.tile_pool(name="grp", bufs=2) as gpool,
            tc.tile_pool(name="edge", bufs=3) as epool,
            tc.tile_pool(name="chk", bufs=2) as kpool,
            tc.tile_pool(name="big", bufs=1) as bpool,
            tc.tile_pool(name="dram", bufs=1, space="DRAM") as dpool,
        ):
            # ---- constants / weights resident in SBUF ----
            iota_t = cpool.tile([128, 128], f32)
            nc.sync.dma_start(out=iota_t[:], in_=dins["iota"][:])

            sel_t = cpool.tile([C, H], bft)
            nc.sync.dma_start(out=sel_t[:], in_=dins["sel"][:])
            w = {}
            for nm in ("WkT", "WvT", "WqT", "WoT", "W1Ta", "W1Tb", "W2Ta", "W2Tb"):
                w[nm] = wpool.tile([C, C], bft, name=nm, tag=nm)
                nc.sync.dma_start(out=w[nm][:], in_=dins[nm][:])
            we_t = wpool.tile([ED + 1, H], bft)
            nc.sync.dma_start(out=we_t[:], in_=dins["WeT"][:])
            vec = {}
            for nm in ("b1a", "b1b", "b2", "g1", "bt1", "g2", "bt2"):
                vec[nm] = wpool.tile([128, 1], f32, name=nm, tag=nm)
                nc.sync.dma_start(out=vec[nm][:], in_=dins[nm][:])
            qT_t = bpool.tile([IN, NSH], f32)
            nc.gpsimd.dma_start(out=qT_t[:], in_=dins["qT"][:])

            # normalized aggregation output, channel-major, bf16
            aggT_sb = bpool.tile([C, NSH], bft)
            ident = cpool.tile([128, 128], bft)
            iota_col = cpool.tile([128, 1], mybir.dt.int32)
            nc.gpsimd.iota(iota_col[:], [[0, 1]], channel_multiplier=1)
            iota_col_f = cpool.tile([128, 1], f32)
            nc.vector.tensor_copy(iota_col_f[:], iota_col[:])
            nc.vector.tensor_tensor(
                out=ident[:], in0=iota_col_f[:].to_broadcast([128, 128]),
                in1=iota_t[:], op=mybir.AluOpType.is_equal)

            # ---- phase 1: per group of 128 dst nodes ----
            ph1 = ExitStack()
            kqpool = ph1.enter_context(tc.tile_pool(name="kqps", bufs=1, space="PSUM"))
            scpool = ph1.enter_context(tc.tile_pool(name="scps", bufs=2, space="PSUM"))
            vppool = ph1.enter_context(tc.tile_pool(name="vpps", bufs=2, space="PSUM"))
            aggpool = ph1.enter_context(tc.tile_pool(name="agg", bufs=1, space="PSUM"))
            txpool = ph1.enter_context(tc.tile_pool(name="txps", bufs=1, space="PSUM"))
            prev_tail = None
            for g in range(NG):
                n_lo = g * 128
                n_hi = min(NSH - n_lo, 128)
                e_g = g * SG
                # batched group DMAs
                kT_g = gpool.tile([IN, SG], mybir.dt.float8e4, tag="kTg")
                vT_g = gpool.tile([IN, SG], bft, tag="vTg")
                qT_g = gpool.tile([IN, SG], mybir.dt.float8e4, tag="qTg")
                efT_g = gpool.tile([ED + 1, SG], bft, tag="efTg")
                ohT_g = gpool.tile([128, SG], mybir.dt.float8e4, tag="ohTg")
                nc.sync.dma_start(out=kT_g[:], in_=dins["kT"][:, e_g:e_g + SG])
                nc.sync.dma_start(out=vT_g[:], in_=dins["vT"][:, e_g:e_g + SG])
                nc.sync.dma_start(out=qT_g[:], in_=dins["qeT"][:, e_g:e_g + SG])
                nc.gpsimd.dma_start(out=efT_g[:], in_=dins["efT"][:, e_g:e_g + SG])
                nc.gpsimd.dma_start(out=ohT_g[:], in_=dins["ohT"][:, e_g:e_g + SG])

                agg_ps = aggpool.tile([128, C + H], f32)

                def front(t0, cw):
                    """Projections + scores + exp for one chunk; returns
                    state needed by back()."""
                    W = cw * 128
                    c0 = t0 * 128
                    kp_ps = kqpool.tile([128, 512], f32, tag="kp")
                    qe_ps = kqpool.tile([128, 512], f32, tag="qe")
                    vp_ps = vppool.tile([128, 512], f32, tag="vp")
                    nc.tensor.matmul(kp_ps[:, 0:W], lhsT=w["WkT"][:],
                                     rhs=kT_g[:, c0:c0 + W], start=True, stop=True)
                    nc.tensor.matmul(qe_ps[:, 0:W], lhsT=w["WqT"][:],
                                     rhs=qT_g[:, c0:c0 + W], start=True, stop=True)
                    for j in range(cw):
                        t = t0 + j
                        nc.tensor.matmul(vp_ps[:, j * C:(j + 1) * C],
                                         lhsT=vT_g[:, t * 128:(t + 1) * 128],
                                         rhs=w["WvT"][:], start=True, stop=True)
                    kp_sb = kpool.tile([128, 512], bft, tag="kpsb")
                    nc.scalar.copy(kp_sb[:, 0:W], kp_ps[:, 0:W])
                    prod = kpool.tile([128, 512], bft, tag="prod")
                    nc.vector.tensor_tensor(
                        out=prod[:, 0:W], in0=qe_ps[:, 0:W], in1=kp_sb[:, 0:W],
                        op=mybir.AluOpType.mult)

                    # per-tile score matmuls into one PSUM tile for the chunk
                    sc_ps = scpool.tile([128, 4 * H], f32, tag="sc")
                    for j in range(cw):
                        t = t0 + j
                        nc.tensor.matmul(
                            sc_ps[:, j * H:(j + 1) * H],
                            lhsT=prod[:, j * 128:(j + 1) * 128],
                            rhs=sel_t[:], start=True, stop=False)
                        nc.tensor.matmul(
                            sc_ps[:, j * H:(j + 1) * H],
                            lhsT=efT_g[:, t * 128:(t + 1) * 128],
                            rhs=we_t[:], start=False, stop=True)
                    # exp over the whole chunk, written into the ex stripes
                    # of the combined [vw | ex] rhs tile: the aggregation
                    # (values + softmax denominators) is then a single
                    # matmul chain per tile into one PSUM bank.  (start=True
                    # clears the whole bank's has_written bits, so two
                    # interleaved chains must never share a bank.)
                    vwex = epool.tile([128, 4 * (C + H)], bft, tag="vwex")
                    vwex3 = vwex[:].rearrange("p (c f) -> p c f", f=C + H)
                    nc.scalar.activation(
                        vwex3[:, 0:cw, C:C + H], sc_ps[:, 0:cw * H],
                        mybir.ActivationFunctionType.Exp)
                    return t0, cw, vp_ps, vwex

                def back(state):
                    """Attention-weighted values + aggregation matmuls."""
                    t0, cw, vp_ps, vwex = state
                    vwex3 = vwex[:].rearrange("p (c f) -> p c f", f=C + H)
                    # vw for the whole chunk in one wide DVE op
                    nc.vector.tensor_tensor(
                        out=vwex3[:, 0:cw, 0:C].rearrange(
                            "p c (h d) -> p c h d", h=H),
                        in0=vp_ps[:, 0:cw * C].rearrange(
                            "p (c h d) -> p c h d", c=cw, h=H),
                        in1=vwex3[:, 0:cw, C:C + H].to_broadcast(
                            [128, cw, H, D]),
                        op=mybir.AluOpType.mult)
                    for j in range(cw):
                        t = t0 + j
                        nc.tensor.matmul(agg_ps[:],
                                         lhsT=ohT_g[:, t * 128:(t + 1) * 128],
                                         rhs=vwex[:, j * (C + H):(j + 1) * (C + H)],
                                         start=(t == 0), stop=(t == SG_TILES - 1))

                # software pipeline: back(c) is emitted after front(c+1) so
                # PE has independent work queued while DVE/ACT fill chunk c.
                # The previous group's normalize/transpose tail is likewise
                # deferred past this group's first front.
                pending = None
                t0 = 0
                for ic, cw in enumerate(CHUNKS):
                    st = front(t0, cw)
                    if ic == 0 and prev_tail is not None:
                        prev_tail()
                        prev_tail = None
                    if pending is not None:
                        back(pending)
                    pending = st
                    t0 += cw
                back(pending)

                def _tail(agg_ps=agg_ps, n_lo=n_lo, n_hi=n_hi):
                    # normalize by denominator, transpose to channel-major
                    rec = epool.tile([128, H], f32, tag="rec")
                    nc.vector.reciprocal(rec[:], agg_ps[:, C:C + H])
                    aggn = epool.tile([128, C], bft, tag="aggn")
                    nc.vector.tensor_tensor(
                        out=aggn[:].rearrange("p (h d) -> p h d", h=H),
                        in0=agg_ps[:, 0:C].rearrange("p (h d) -> p h d", h=H),
                        in1=rec[:].to_broadcast([128, H, D]),
                        op=mybir.AluOpType.mult)
                    aggnT_ps = txpool.tile([128, 128], bft, tag="aggT")
                    nc.tensor.transpose(aggnT_ps[:], aggn[:], ident[:])
                    nc.scalar.copy(aggT_sb[:, n_lo:n_lo + n_hi],
                                   aggnT_ps[:, 0:n_hi])
                prev_tail = _tail

            prev_tail()
            ph1.close()
            # ---- phase 2: channel-major dense, BN stats fused per chunk ----
            p2ctx = ExitStack()
            p2pool = p2ctx.enter_context(tc.tile_pool(name="ph2ps", bufs=2, space="PSUM"))

            def stats_accum(parts, ci, x_chunk, tag):
                # per-chunk partial sum/sumsq into distinct columns; summed
                # once at the end (no serial accumulation chain)
                nc.vector.tensor_reduce(out=parts[:, ci:ci + 1], in_=x_chunk,
                                        axis=mybir.AxisListType.X,
                                        op=mybir.AluOpType.add)
                sqs = epool.tile([128, CH], bft, tag=f"sq{tag}")
                nc.scalar.activation(sqs[:], x_chunk,
                                     mybir.ActivationFunctionType.Square,
                                     accum_out=parts[:, NCH + ci:NCH + ci + 1])

            def stats_total(parts, st):
                nc.vector.tensor_reduce(out=st[:, 0:1], in_=parts[:, 0:NCH],
                                        axis=mybir.AxisListType.X,
                                        op=mybir.AluOpType.add)
                nc.vector.tensor_reduce(out=st[:, 1:2],
                                        in_=parts[:, NCH:2 * NCH],
                                        axis=mybir.AxisListType.X,
                                        op=mybir.AluOpType.add)

            rst = bpool.tile([C, NSH], f32)
            st1 = bpool.tile([128, 2], f32, tag="st1")
            parts1 = bpool.tile([128, 2 * NCH], f32, tag="parts1")
            for ci in range(NCH):
                s0_ = ci * CH
                ps = p2pool.tile([128, CH], f32, tag="wo")
                nc.tensor.matmul(ps[:], lhsT=w["WoT"][:],
                                 rhs=aggT_sb[:, s0_:s0_ + CH], start=True, stop=True)
                nc.vector.tensor_tensor(out=rst[:, s0_:s0_ + CH], in0=ps[:],
                                        in1=qT_t[:, s0_:s0_ + CH],
                                        op=mybir.AluOpType.add)
                stats_accum(parts1, ci, rst[:, s0_:s0_ + CH], "1")
            stats_total(parts1, st1)

            def bn_finalize(st, gv, btv, suffix):
                # global mean/var across all N nodes (AllReduce of sum/sumsq)
                bounce_in = dpool.tile([128, 2], f32, tag=f"bi{suffix}")
                bounce_out = dpool.tile([128, 2], f32, tag=f"bo{suffix}")
                nc.gpsimd.dma_start(out=bounce_in[:], in_=st[:])
                nc.gpsimd.collective_compute(
                    "AllReduce", mybir.AluOpType.add,
                    replica_groups=[list(range(NCORE))],
                    ins=[bounce_in.opt()], outs=[bounce_out.opt()])
                stg = bpool.tile([128, 2], f32, tag=f"stg{suffix}")
                nc.sync.dma_start(out=stg[:], in_=bounce_out[:])
                mean = bpool.tile([128, 1], f32, tag=f"mean{suffix}")
                nc.vector.tensor_scalar_mul(mean[:], stg[:, 0:1], 1.0 / N)
                msq = bpool.tile([128, 1], f32, tag=f"msq{suffix}")
                nc.scalar.activation(msq[:], mean[:],
                                     mybir.ActivationFunctionType.Square)
                var = bpool.tile([128, 1], f32, tag=f"var{suffix}")
                nc.vector.tensor_scalar_mul(var[:], stg[:, 1:2], 1.0 / N)
                nc.vector.tensor_tensor(out=var[:], in0=var[:], in1=msq[:],
                                        op=mybir.AluOpType.subtract)
                nc.vector.tensor_scalar_add(var[:], var[:], float(EPS))
                sd = bpool.tile([128, 1], f32, tag=f"sd{suffix}")
                nc.scalar.activation(sd[:], var[:],
                                     mybir.ActivationFunctionType.Sqrt)
                rsd = bpool.tile([128, 1], f32, tag=f"rsd{suffix}")
                nc.vector.reciprocal(rsd[:], sd[:])
                scale = bpool.tile([128, 1], f32, tag=f"scale{suffix}")
                nc.vector.tensor_tensor(out=scale[:], in0=rsd[:], in1=gv[:],
                                        op=mybir.AluOpType.mult)
                nmean = bpool.tile([128, 1], f32, tag=f"nm{suffix}")
                nc.vector.tensor_tensor(out=nmean[:], in0=mean[:], in1=scale[:],
                                        op=mybir.AluOpType.mult)
                shift = bpool.tile([128, 1], f32, tag=f"shift{suffix}")
                nc.vector.tensor_tensor(out=shift[:], in0=btv[:], in1=nmean[:],
                                        op=mybir.AluOpType.subtract)
                return scale, shift

            sc1, sh1 = bn_finalize(st1, vec["g1"], vec["bt1"], "1")
            xbn_bf = bpool.tile([C, NSH], bft)
            for ci in range(NCH):
                s0_ = ci * CH
                nc.scalar.activation(xbn_bf[:, s0_:s0_ + CH],
                                     rst[:, s0_:s0_ + CH],
                                     mybir.ActivationFunctionType.Identity,
                                     bias=sh1[:], scale=sc1[:])
            st2 = bpool.tile([128, 2], f32, tag="st2")
            parts2 = bpool.tile([128, 2 * NCH], f32, tag="parts2")
            y = bpool.tile([C, NSH], f32)
            for ci in range(NCH):
                s0_ = ci * CH
                rhs2 = xbn_bf[:, s0_:s0_ + CH]
                h1a = p2pool.tile([128, CH], f32, tag="h1a")
                h1b = p2pool.tile([128, CH], f32, tag="h1b")
                nc.tensor.matmul(h1a[:], lhsT=w["W1Ta"][:], rhs=rhs2, start=True, stop=True)
                nc.tensor.matmul(h1b[:], lhsT=w["W1Tb"][:], rhs=rhs2, start=True, stop=True)
                r1a = epool.tile([128, CH], bft, tag="r1a")
                r1b = epool.tile([128, CH], bft, tag="r1b")
                nc.scalar.activation(r1a[:], h1a[:],
                                     mybir.ActivationFunctionType.Relu,
                                     bias=vec["b1a"][:])
                nc.scalar.activation(r1b[:], h1b[:],
                                     mybir.ActivationFunctionType.Relu,
                                     bias=vec["b1b"][:])
                h2 = p2pool.tile([128, CH], f32, tag="h2")
                nc.tensor.matmul(h2[:], lhsT=w["W2Ta"][:], rhs=r1a[:], start=True, stop=False)
                nc.tensor.matmul(h2[:], lhsT=w["W2Tb"][:], rhs=r1b[:], start=False, stop=True)
                # y = h2 + b2 + xbn
                yt = epool.tile([128, CH], f32, tag="yt")
                nc.scalar.activation(yt[:], h2[:],
                                     mybir.ActivationFunctionType.Identity,
                                     bias=vec["b2"][:])
                nc.vector.tensor_tensor(out=y[:, s0_:s0_ + CH], in0=yt[:],
                                        in1=rhs2,
                                        op=mybir.AluOpType.add)
                stats_accum(parts2, ci, y[:, s0_:s0_ + CH], "2")

            stats_total(parts2, st2)
            sc2, sh2 = bn_finalize(st2, vec["g2"], vec["bt2"], "2")
            for ci in range(NCH):
                s0_ = ci * CH
                yo = epool.tile([128, CH], f32, tag="yo")
                nc.scalar.activation(yo[:], y[:, s0_:s0_ + CH],
                                     mybir.ActivationFunctionType.Identity,
                                     bias=sh2[:], scale=sc2[:])
                nc.sync.dma_start(out=dout[:, s0_:s0_ + CH], in_=yo[:])
            p2ctx.close()
    return nc


def _host_prep(q, k, v, edge_feat, src, dst, Wq, Wk, Wv, We, be, Wo,
               W1, b1, W2, b2, g1, bt1, g2, bt2):
    order = np.argsort(dst, kind="stable")
    src_s = src[order]
    dst_s = dst[order]
    ef_s = edge_feat[order]

    sel = np.zeros((C, H), dtype=bf16)
    for h in range(H):
        sel[h * D:(h + 1) * D, h] = 1.0

    in_maps = []
    for m in range(NCORE):
        lo, hi = m * NSH, (m + 1) * NSH
        selm = (dst_s >= lo) & (dst_s < hi)
        srcm, dstm, efm = src_s[selm], dst_s[selm] - lo, ef_s[selm]
        # slot layout: per group g, SG slots
        fp8 = ml_dtypes.float8_e4m3
        kT = np.zeros((IN, S), dtype=fp8)
        vT = np.zeros((IN, S), dtype=bf16)
        qeT = np.zeros((IN, S), dtype=fp8)
        efT = np.zeros((ED + 1, S), dtype=bf16)
        dstrel = np.full((128, NG * SG_TILES), -1.0, dtype=np.float32)
        grp = dstm // 128
        for g in range(NG):
            gs = np.nonzero(grp == g)[0]
            ne = len(gs)
            assert ne <= SG, f"group {g} core {m} has {ne} edges > SG={SG}"
            base = g * SG
            kT[:, base:base + ne] = k[srcm[gs]].T
            vT[:, base:base + ne] = v[srcm[gs]].T
            qeT[:, base:base + ne] = q[dstm[gs] + lo].T
            efT[:ED, base:base + ne] = efm[gs].T
            efT[ED, base:base + ne] = 1.0
            rel = (dstm[gs] - g * 128).astype(np.float32)
            sl = np.arange(ne)
            dstrel[sl % 128, g * SG_TILES + sl // 128] = rel
        # one-hot [slot%128, n] per tile, precomputed in fp8 (0/1 exact)
        ohT = (dstrel.reshape(128, NG * SG_TILES, 1)
               == np.arange(128, dtype=np.float32)[None, None, :])
        ohT = ohT.reshape(128, S).astype(ml_dtypes.float8_e4m3)
        iota = np.broadcast_to(np.arange(128, dtype=np.float32), (128, 128)).copy()
        im = {
            "kT": kT, "vT": vT, "qeT": qeT, "efT": efT,
            "ohT": ohT, "iota": iota, "sel": sel.copy(),
            "qT": q[lo:hi].T.astype(np.float32).copy(),
            "WkT": Wk.T.astype(bf16).copy(),
            "WvT": Wv.T.astype(bf16).copy(),
            "WqT": (Wq / np.sqrt(np.float32(D))).T.astype(bf16).copy(),
            "WeT": np.concatenate([We.T, be[None, :]], 0).astype(bf16).copy(),
            "WoT": Wo.T.astype(bf16).copy(),
            "W1Ta": W1[:C].T.astype(bf16).copy(),
            "W1Tb": W1[C:].T.astype(bf16).copy(),
            "W2Ta": W2.T[:C].astype(bf16).copy(),
            "W2Tb": W2.T[C:].astype(bf16).copy(),
            "b1a": b1[:C, None].astype(np.float32).copy(),
            "b1b": b1[C:, None].astype(np.float32).copy(),
            "b2": b2[:, None].astype(np.float32).copy(),
            "g1": g1[:, None].astype(np.float32).copy(),
            "bt1": bt1[:, None].astype(np.float32).copy(),
            "g2": g2[:, None].astype(np.float32).copy(),
            "bt2": bt2[:, None].astype(np.float32).copy(),
        }
        in_maps.append(im)
    return in_maps


RUN_KW = {}
LAST = {}


def kernel(**inputs):
    inputs = {k: np.asarray(v) for k, v in inputs.items()}
    in_maps = _host_prep(**inputs)
    nc = _build_program()
    res = run_bass_kernel_spmd(nc, in_maps, core_ids=list(range(NCORE)),
                               **RUN_KW)
    LAST["res"] = res
    out = np.concatenate([r["out"].T for r in res.results], axis=0)
    return out.astype(np.float32)


# revision 48
# speedup vs baseline: 1.0376x; 1.0376x over previous
"""GTLayer (graph transformer layer) distributed Bass kernel for 8 TRN2 cores.

Sharding: nodes (and their incoming edges) are partitioned across 8 cores by
node id (5000 dst nodes/core).  Host-side prep redistributes RAW input rows
per edge (the halo-exchange analog): for each core's dst-sorted, padded edge
list we build transposed per-edge arrays kT[e]=k[src_e], vT, qeT(dst), efT.

Device pipeline (per core), per group of 128 dst nodes (17 edge tiles):
 - 5 batched DMAs bring the group's edge streams into SBUF.
 - Per 512-edge chunk: weight-stationary matmuls produce channel-major
   kp_cm/qe_cm [C, 512] in PSUM; ACT copies kp->SBUF bf16; DVE forms
   prod_cm = qe*kp (bf16).
 - Per 128-edge tile: PE contracts prod_cm slice against a constant
   head-selector [C,H] and accumulates the edge-bias matmul into the same
   PSUM scores tile; ACT exps 4 tiles of scores at once; DVE weights vp by
   attn; PE aggregates (one-hot matmul) values+denominators per group.
 - GpSimd builds the one-hot tiles (is_equal vs iota) off the critical path.
Then Wo + residual + BN (global stats via AllReduce) + FFN + BN, identical
to the reference semantics.
"""

import json
from contextlib import ExitStack
import numpy as np
import ml_dtypes

import concourse.bass as bass
import concourse.mybir as mybir
import concourse.tile as tile
from concourse.bass_utils import run_bass_kernel_spmd

bf16 = ml_dtypes.bfloat16

# problem constants (hardcoded per contract)
N, E, IN, H, D, ED = 40000, 640000, 128, 8, 16, 64
C = H * D            # 128
NCORE = 8
NSH = N // NCORE     # 5000 nodes per core
NG = 40              # node groups of <=128 per core (39*128+8)
SG_TILES = 17        # padded edge tiles per group (17*128 = 2176 slots)
SG = SG_TILES * 128
S = NG * SG          # slots per core
EPS = 1e-5
# chunk structure within a group: 4 chunks of 4 tiles + 1 odd tile
CHUNKS = [4, 4, 4, 4, 1]
assert sum(CHUNKS) == SG_TILES

f32 = mybir.dt.float32
bft = mybir.dt.bfloat16


def _split_multiwaits_json(bir: bytes) -> bytes:
    """This walrus build allows only ONE sem wait per instruction; Tile emits
    multi-waits.  Split extras onto NoOps inserted before, same engine."""
    b = json.loads(bir)
    ctr = [0]
    changed = False
    for f in b.get("functions", []):
        for blk in f.get("blocks", []):
            insts = blk.get("instructions")
            if not insts:
                continue
            out = []
            for i in insts:
                si = i.get("sync_info")
                waits = (si or {}).get("on_wait") or []
                if len(waits) > 1:
                    changed = True
                    for w in waits[:-1]:
                        ctr[0] += 1
                        out.append({
                            "debug": i.get("debug", 0), "engine": i["engine"],
                            "ins": [], "name": f"I-wsplit-{ctr[0]}",
                            "opcode": "NoOp", "outs": [],
                            "text_hint": "wsplit",
                            "sync_info": {"on_update": [], "on_wait": [w]},
                        })
                    si["on_wait"] = [waits[-1]]
                out.append(i)
            blk["instructions"] = out
    return json.dumps(b).encode() if changed else bir


class _BassW(bass.Bass):
    def to_json_bytes(self) -> bytes:
        return _split_multiwaits_json(super().to_json_bytes())


def _build_program():
    nc = _BassW()
    dt_in = {
        "kT": (mybir.dt.float8e4, [IN, S]), "vT": (bft, [IN, S]),
        "qeT": (mybir.dt.float8e4, [IN, S]),
        "efT": (bft, [ED + 1, S]),
        "ohT": (mybir.dt.float8e4, [128, S]),
        "iota": (f32, [128, 128]),
        "sel": (bft, [C, H]),
        "qT": (f32, [IN, NSH]),
        "WkT": (bft, [IN, C]), "WvT": (bft, [IN, C]), "WqT": (bft, [IN, C]),
        "WeT": (bft, [ED + 1, H]),
        "WoT": (bft, [C, C]),
        "W1Ta": (bft, [C, C]), "W1Tb": (bft, [C, C]),
        "W2Ta": (bft, [C, C]), "W2Tb": (bft, [C, C]),
        "b1a": (f32, [128, 1]), "b1b": (f32, [128, 1]), "b2": (f32, [128, 1]),
        "g1": (f32, [128, 1]), "bt1": (f32, [128, 1]),
        "g2": (f32, [128, 1]), "bt2": (f32, [128, 1]),
    }
    dins = {k: nc.dram_tensor(k, sh, dt, kind="ExternalInput")
            for k, (dt, sh) in dt_in.items()}
    dout = nc.dram_tensor("out", [C, NSH], f32, kind="ExternalOutput")

    CH = 500  # phase-2 node chunk
    NCH = NSH // CH

    with tile.TileContext(nc) as tc:
        with (
            tc.tile_pool(name="const", bufs=1) as cpool,
            tc.tile_pool(name="wts", bufs=1) as wpool,
            tc.tile_pool(name="grp", bufs=3) as gpool,
            tc.tile_pool(name="edge", bufs=3) as epool,
            tc.tile_pool(name="chk", bufs=2) as kpool,
            tc.tile_pool(name="big", bufs=1) as bpool,
            tc.tile_pool(name="dram", bufs=1, space="DRAM") as dpool,
        ):
            # ---- constants / weights resident in SBUF ----
            iota_t = cpool.tile([128, 128], f32)
            nc.sync.dma_start(out=iota_t[:], in_=dins["iota"][:])

            sel_t = cpool.tile([C, H], bft)
            nc.sync.dma_start(out=sel_t[:], in_=dins["sel"][:])
            w = {}
            for nm in ("WkT", "WvT", "WqT", "WoT", "W1Ta", "W1Tb", "W2Ta", "W2Tb"):
                w[nm] = wpool.tile([C, C], bft, name=nm, tag=nm)
                nc.sync.dma_start(out=w[nm][:], in_=dins[nm][:])
            we_t = wpool.tile([ED + 1, H], bft)
            nc.sync.dma_start(out=we_t[:], in_=dins["WeT"][:])
            vec = {}
            for nm in ("b1a", "b1b", "b2", "g1", "bt1", "g2", "bt2"):
                vec[nm] = wpool.tile([128, 1], f32, name=nm, tag=nm)
                nc.sync.dma_start(out=vec[nm][:], in_=dins[nm][:])
            qT_t = bpool.tile([IN, NSH], f32)
            nc.gpsimd.dma_start(out=qT_t[:], in_=dins["qT"][:])

            # normalized aggregation output, channel-major, bf16
            aggT_sb = bpool.tile([C, NSH], bft)
            ident = cpool.tile([128, 128], bft)
            iota_col = cpool.tile([128, 1], mybir.dt.int32)
            nc.gpsimd.iota(iota_col[:], [[0, 1]], channel_multiplier=1)
            iota_col_f = cpool.tile([128, 1], f32)
            nc.vector.tensor_copy(iota_col_f[:], iota_col[:])
            nc.vector.tensor_tensor(
                out=ident[:], in0=iota_col_f[:].to_broadcast([128, 128]),
                in1=iota_t[:], op=mybir.AluOpType.is_equal)

            # ---- phase 1: per group of 128 dst nodes ----
            ph1 = ExitStack()
            kqpool = ph1.enter_context(tc.tile_pool(name="kqps", bufs=1, space="PSUM"))
            scpool = ph1.enter_context(tc.tile_pool(name="scps", bufs=2, space="PSUM"))
            vppool = ph1.enter_context(tc.tile_pool(name="vpps", bufs=2, space="PSUM"))
            aggpool = ph1.enter_context(tc.tile_pool(name="agg", bufs=1, space="PSUM"))
            txpool = ph1.enter_context(tc.tile_pool(name="txps", bufs=1, space="PSUM"))
            prev_tail = None
            for g in range(NG):
                n_lo = g * 128
                n_hi = min(NSH - n_lo, 128)
                e_g = g * SG
                # batched group DMAs
                kT_g = gpool.tile([IN, SG], mybir.dt.float8e4, tag="kTg")
                vT_g = gpool.tile([IN, SG], bft, tag="vTg")
                qT_g = gpool.tile([IN, SG], mybir.dt.float8e4, tag="qTg")
                efT_g = gpool.tile([ED + 1, SG], bft, tag="efTg")
                ohT_g = gpool.tile([128, SG], mybir.dt.float8e4, tag="ohTg")
                nc.sync.dma_start(out=kT_g[:], in_=dins["kT"][:, e_g:e_g + SG])
                nc.sync.dma_start(out=vT_g[:], in_=dins["vT"][:, e_g:e_g + SG])
                nc.sync.dma_start(out=qT_g[:], in_=dins["qeT"][:, e_g:e_g + SG])
                nc.gpsimd.dma_start(out=efT_g[:], in_=dins["efT"][:, e_g:e_g + SG])
                nc.gpsimd.dma_start(out=ohT_g[:], in_=dins["ohT"][:, e_g:e_g + SG])

                agg_ps = aggpool.tile([128, C + H], f32)

                def front(t0, cw):
                    """Projections + scores + exp for one chunk; returns
                    state needed by back()."""
                    W = cw * 128
                    c0 = t0 * 128
                    kp_ps = kqpool.tile([128, 512], f32, tag="kp")
                    qe_ps = kqpool.tile([128, 512], f32, tag="qe")
                    vp_ps = vppool.tile([128, 512], f32, tag="vp")
                    nc.tensor.matmul(kp_ps[:, 0:W], lhsT=w["WkT"][:],
                                     rhs=kT_g[:, c0:c0 + W], start=True, stop=True)
                    nc.tensor.matmul(qe_ps[:, 0:W], lhsT=w["WqT"][:],
                                     rhs=qT_g[:, c0:c0 + W], start=True, stop=True)
                    for j in range(cw):
                        t = t0 + j
                        nc.tensor.matmul(vp_ps[:, j * C:(j + 1) * C],
                                         lhsT=vT_g[:, t * 128:(t + 1) * 128],
                                         rhs=w["WvT"][:], start=True, stop=True)
                    kp_sb = kpool.tile([128, 512], bft, tag="kpsb")
                    nc.scalar.copy(kp_sb[:, 0:W], kp_ps[:, 0:W])
                    prod = kpool.tile([128, 512], bft, tag="prod")
                    nc.vector.tensor_tensor(
                        out=prod[:, 0:W], in0=qe_ps[:, 0:W], in1=kp_sb[:, 0:W],
                        op=mybir.AluOpType.mult)

                    # per-tile score matmuls into one PSUM tile for the chunk
                    sc_ps = scpool.tile([128, 4 * H], f32, tag="sc")
                    for j in range(cw):
                        t = t0 + j
                        nc.tensor.matmul(
                            sc_ps[:, j * H:(j + 1) * H],
                            lhsT=prod[:, j * 128:(j + 1) * 128],
                            rhs=sel_t[:], start=True, stop=False)
                        nc.tensor.matmul(
                            sc_ps[:, j * H:(j + 1) * H],
                            lhsT=efT_g[:, t * 128:(t + 1) * 128],
                            rhs=we_t[:], start=False, stop=True)
                    # exp over the whole chunk, written into the ex stripes
                    # of the combined [vw | ex] rhs tile: the aggregation
                    # (values + softmax denominators) is then a single
                    # matmul chain per tile into one PSUM bank.  (start=True
                    # clears the whole bank's has_written bits, so two
                    # interleaved chains must never share a bank.)
                    vwex = epool.tile([128, 4 * (C + H)], bft, tag="vwex")
                    vwex3 = vwex[:].rearrange("p (c f) -> p c f", f=C + H)
                    nc.scalar.activation(
                        vwex3[:, 0:cw, C:C + H], sc_ps[:, 0:cw * H],
                        mybir.ActivationFunctionType.Exp)
                    return t0, cw, vp_ps, vwex

                def back(state):
                    """Attention-weighted values + aggregation matmuls."""
                    t0, cw, vp_ps, vwex = state
                    vwex3 = vwex[:].rearrange("p (c f) -> p c f", f=C + H)
                    # vw for the whole chunk in one wide DVE op
                    nc.vector.tensor_tensor(
                        out=vwex3[:, 0:cw, 0:C].rearrange(
                            "p c (h d) -> p c h d", h=H),
                        in0=vp_ps[:, 0:cw * C].rearrange(
                            "p (c h d) -> p c h d", c=cw, h=H),
                        in1=vwex3[:, 0:cw, C:C + H].to_broadcast(
                            [128, cw, H, D]),
                        op=mybir.AluOpType.mult)
                    for j in range(cw):
                        t = t0 + j
                        nc.tensor.matmul(agg_ps[:],
                                         lhsT=ohT_g[:, t * 128:(t + 1) * 128],
                                         rhs=vwex[:, j * (C + H):(j + 1) * (C + H)],
                                         start=(t == 0), stop=(t == SG_TILES - 1))

                # software pipeline: back(c) is emitted after front(c+1) so
                # PE has independent work queued while DVE/ACT fill chunk c.
                # The previous group's normalize/transpose tail is likewise
                # deferred past this group's first front.
                pending = None
                t0 = 0
                for ic, cw in enumerate(CHUNKS):
                    st = front(t0, cw)
                    if ic == 0 and prev_tail is not None:
                        prev_tail()
                        prev_tail = None
                    if pending is not None:
                        back(pending)
                    pending = st
                    t0 += cw
                back(pending)

                def _tail(agg_ps=agg_ps, n_lo=n_lo, n_hi=n_hi):
                    # normalize by denominator, transpose to channel-major
                    rec = epool.tile([128, H], f32, tag="rec")
                    nc.vector.reciprocal(rec[:], agg_ps[:, C:C + H])
                    aggn = epool.tile([128, C], bft, tag="aggn")
                    nc.vector.tensor_tensor(
                        out=aggn[:].rearrange("p (h d) -> p h d", h=H),
                        in0=agg_ps[:, 0:C].rearrange("p (h d) -> p h d", h=H),
                        in1=rec[:].to_broadcast([128, H, D]),
                        op=mybir.AluOpType.mult)
                    aggnT_ps = txpool.tile([128, 128], bft, tag="aggT")
                    nc.tensor.transpose(aggnT_ps[:], aggn[:], ident[:])
                    nc.scalar.copy(aggT_sb[:, n_lo:n_lo + n_hi],
                                   aggnT_ps[:, 0:n_hi])
                prev_tail = _tail

            prev_tail()
            ph1.close()
            # ---- phase 2: channel-major dense, BN stats fused per chunk ----
            p2ctx = ExitStack()
            p2pool = p2ctx.enter_context(tc.tile_pool(name="ph2ps", bufs=2, space="PSUM"))

            def stats_accum(parts, ci, x_chunk, tag):
                # per-chunk partial sum/sumsq into distinct columns; summed
                # once at the end (no serial accumulation chain)
                nc.vector.tensor_reduce(out=parts[:, ci:ci + 1], in_=x_chunk,
                                        axis=mybir.AxisListType.X,
                                        op=mybir.AluOpType.add)
                sqs = epool.tile([128, CH], bft, tag=f"sq{tag}")
                nc.scalar.activation(sqs[:], x_chunk,
                                     mybir.ActivationFunctionType.Square,
                                     accum_out=parts[:, NCH + ci:NCH + ci + 1])

            def stats_total(parts, st):
                nc.vector.tensor_reduce(out=st[:, 0:1], in_=parts[:, 0:NCH],
                                        axis=mybir.AxisListType.X,
                                        op=mybir.AluOpType.add)
                nc.vector.tensor_reduce(out=st[:, 1:2],
                                        in_=parts[:, NCH:2 * NCH],
                                        axis=mybir.AxisListType.X,
                                        op=mybir.AluOpType.add)

            rst = bpool.tile([C, NSH], f32)
            st1 = bpool.tile([128, 2], f32, tag="st1")
            parts1 = bpool.tile([128, 2 * NCH], f32, tag="parts1")
            for ci in range(NCH):
                s0_ = ci * CH
                ps = p2pool.tile([128, CH], f32, tag="wo")
                nc.tensor.matmul(ps[:], lhsT=w["WoT"][:],
                                 rhs=aggT_sb[:, s0_:s0_ + CH], start=True, stop=True)
                nc.vector.tensor_tensor(out=rst[:, s0_:s0_ + CH], in0=ps[:],
                                        in1=qT_t[:, s0_:s0_ + CH],
                                        op=mybir.AluOpType.add)
                stats_accum(parts1, ci, rst[:, s0_:s0_ + CH], "1")
            stats_total(parts1, st1)

            def bn_finalize(st, gv, btv, suffix):
                # global mean/var across all N nodes (AllReduce of sum/sumsq)
                bounce_in = dpool.tile([128, 2], f32, tag=f"bi{suffix}")
                bounce_out = dpool.tile([128, 2], f32, tag=f"bo{suffix}")
                nc.gpsimd.dma_start(out=bounce_in[:], in_=st[:])
                nc.gpsimd.collective_compute(
                    "AllReduce", mybir.AluOpType.add,
                    replica_groups=[list(range(NCORE))],
                    ins=[bounce_in.opt()], outs=[bounce_out.opt()])
                stg = bpool.tile([128, 2], f32, tag=f"stg{suffix}")
                nc.sync.dma_start(out=stg[:], in_=bounce_out[:])
                mean = bpool.tile([128, 1], f32, tag=f"mean{suffix}")
                nc.vector.tensor_scalar_mul(mean[:], stg[:, 0:1], 1.0 / N)
                msq = bpool.tile([128, 1], f32, tag=f"msq{suffix}")
                nc.scalar.activation(msq[:], mean[:],
                                     mybir.ActivationFunctionType.Square)
                var = bpool.tile([128, 1], f32, tag=f"var{suffix}")
                nc.vector.tensor_scalar_mul(var[:], stg[:, 1:2], 1.0 / N)
                nc.vector.tensor_tensor(out=var[:], in0=var[:], in1=msq[:],
                                        op=mybir.AluOpType.subtract)
                nc.vector.tensor_scalar_add(var[:], var[:], float(EPS))
                sd = bpool.tile([128, 1], f32, tag=f"sd{suffix}")
                nc.scalar.activation(sd[:], var[:],
                                     mybir.ActivationFunctionType.Sqrt)
                rsd = bpool.tile([128, 1], f32, tag=f"rsd{suffix}")
                nc.vector.reciprocal(rsd[:], sd[:])
                scale = bpool.tile([128, 1], f32, tag=f"scale{suffix}")
                nc.vector.tensor_tensor(out=scale[:], in0=rsd[:], in1=gv[:],
                                        op=mybir.AluOpType.mult)
                nmean = bpool.tile([128, 1], f32, tag=f"nm{suffix}")
                nc.vector.tensor_tensor(out=nmean[:], in0=mean[:], in1=scale[:],
                                        op=mybir.AluOpType.mult)
                shift = bpool.tile([128, 1], f32, tag=f"shift{suffix}")
                nc.vector.tensor_tensor(out=shift[:], in0=btv[:], in1=nmean[:],
                                        op=mybir.AluOpType.subtract)
                return scale, shift

            sc1, sh1 = bn_finalize(st1, vec["g1"], vec["bt1"], "1")
            xbn_bf = bpool.tile([C, NSH], bft)
            for ci in range(NCH):
                s0_ = ci * CH
                nc.scalar.activation(xbn_bf[:, s0_:s0_ + CH],
                                     rst[:, s0_:s0_ + CH],
                                     mybir.ActivationFunctionType.Identity,
                                     bias=sh1[:], scale=sc1[:])
            st2 = bpool.tile([128, 2], f32, tag="st2")
            parts2 = bpool.tile([128, 2 * NCH], f32, tag="parts2")
            y = bpool.tile([C, NSH], f32)
            for ci in range(NCH):
                s0_ = ci * CH
                rhs2 = xbn_bf[:, s0_:s0_ + CH]
                h1a = p2pool.tile([128, CH], f32, tag="h1a")
                h1b = p2pool.tile([128, CH], f32, tag="h1b")
                nc.tensor.matmul(h1a[:], lhsT=w["W1Ta"][:], rhs=rhs2, start=True, stop=True)
                nc.tensor.matmul(h1b[:], lhsT=w["W1Tb"][:], rhs=rhs2, start=True, stop=True)
                r1a = epool.tile([128, CH], bft, tag="r1a")
                r1b = epool.tile([128, CH], bft, tag="r1b")
                nc.scalar.activation(r1a[:], h1a[:],
                                     mybir.ActivationFunctionType.Relu,
                                     bias=vec["b1a"][:])
                nc.scalar.activation(r1b[:], h1b[:],
                                     mybir.ActivationFunctionType.Relu,
                                     bias=vec["b1b"][:])
                h2 = p2pool.tile([128, CH], f32, tag="h2")
                nc.tensor.matmul(h2[:], lhsT=w["W2Ta"][:], rhs=r1a[:], start=True, stop=False)
                nc.tensor.matmul(h2[:], lhsT=w["W2Tb"][:], rhs=r1b[:], start=False, stop=True)
                # y = h2 + b2 + xbn
                yt = epool.tile([128, CH], f32, tag="yt")
                nc.scalar.activation(yt[:], h2[:],
                                     mybir.ActivationFunctionType.Identity,
                                     bias=vec["b2"][:])
                nc.vector.tensor_tensor(out=y[:, s0_:s0_ + CH], in0=yt[:],
                                        in1=rhs2,
                                        op=mybir.AluOpType.add)
                stats_accum(parts2, ci, y[:, s0_:s0_ + CH], "2")

            stats_total(parts2, st2)
            sc2, sh2 = bn_finalize(st2, vec["g2"], vec["bt2"], "2")
            for ci in range(NCH):
                s0_ = ci * CH
                yo = epool.tile([128, CH], f32, tag="yo")
                nc.scalar.activation(yo[:], y[:, s0_:s0_ + CH],
                                     mybir.ActivationFunctionType.Identity,
                                     bias=sh2[:], scale=sc2[:])
                nc.sync.dma_start(out=dout[:, s0_:s0_ + CH], in_=yo[:])
            p2ctx.close()
    return nc


def _host_prep(q, k, v, edge_feat, src, dst, Wq, Wk, Wv, We, be, Wo,
               W1, b1, W2, b2, g1, bt1, g2, bt2):
    order = np.argsort(dst, kind="stable")
    src_s = src[order]
    dst_s = dst[order]
    ef_s = edge_feat[order]

    sel = np.zeros((C, H), dtype=bf16)
    for h in range(H):
        sel[h * D:(h + 1) * D, h] = 1.0

    in_maps = []
    for m in range(NCORE):
        lo, hi = m * NSH, (m + 1) * NSH
        selm = (dst_s >= lo) & (dst_s < hi)
        srcm, dstm, efm = src_s[selm], dst_s[selm] - lo, ef_s[selm]
        # slot layout: per group g, SG slots
        fp8 = ml_dtypes.float8_e4m3
        kT = np.zeros((IN, S), dtype=fp8)
        vT = np.zeros((IN, S), dtype=bf16)
        qeT = np.zeros((IN, S), dtype=fp8)
        efT = np.zeros((ED + 1, S), dtype=bf16)
        dstrel = np.full((128, NG * SG_TILES), -1.0, dtype=np.float32)
        grp = dstm // 128
        for g in range(NG):
            gs = np.nonzero(grp == g)[0]
            ne = len(gs)
            assert ne <= SG, f"group {g} core {m} has {ne} edges > SG={SG}"
            base = g * SG
            kT[:, base:base + ne] = k[srcm[gs]].T
            vT[:, base:base + ne] = v[srcm[gs]].T
            qeT[:, base:base + ne] = q[dstm[gs] + lo].T
            efT[:ED, base:base + ne] = efm[gs].T
            efT[ED, base:base + ne] = 1.0
            rel = (dstm[gs] - g * 128).astype(np.float32)
            sl = np.arange(ne)
            dstrel[sl % 128, g * SG_TILES + sl // 128] = rel
        # one-hot [slot%128, n] per tile, precomputed in fp8 (0/1 exact)
        ohT = (dstrel.reshape(128, NG * SG_TILES, 1)
               == np.arange(128, dtype=np.float32)[None, None, :])
        ohT = ohT.reshape(128, S).astype(ml_dtypes.float8_e4m3)
        iota = np.broadcast_to(np.arange(128, dtype=np.float32), (128, 128)).copy()
        im = {
            "kT": kT, "vT": vT, "qeT": qeT, "efT": efT,
            "ohT": ohT, "iota": iota, "sel": sel.copy(),
            "qT": q[lo:hi].T.astype(np.float32).copy(),
            "WkT": Wk.T.astype(bf16).copy(),
            "WvT": Wv.T.astype(bf16).copy(),
            "WqT": (Wq / np.sqrt(np.float32(D))).T.astype(bf16).copy(),
            "WeT": np.concatenate([We.T, be[None, :]], 0).astype(bf16).copy(),
            "WoT": Wo.T.astype(bf16).copy(),
            "W1Ta": W1[:C].T.astype(bf16).copy(),
            "W1Tb": W1[C:].T.astype(bf16).copy(),
            "W2Ta": W2.T[:C].astype(bf16).copy(),
            "W2Tb": W2.T[C:].astype(bf16).copy(),
            "b1a": b1[:C, None].astype(np.float32).copy(),
            "b1b": b1[C:, None].astype(np.float32).copy(),
            "b2": b2[:, None].astype(np.float32).copy(),
            "g1": g1[:, None].astype(np.float32).copy(),
            "bt1": bt1[:, None].astype(np.float32).copy(),
            "g2": g2[:, None].astype(np.float32).copy(),
            "bt2": bt2[:, None].astype(np.float32).copy(),
        }
        in_maps.append(im)
    return in_maps


RUN_KW = {}
LAST = {}


def kernel(**inputs):
    inputs = {k: np.asarray(v) for k, v in inputs.items()}
    in_maps = _host_prep(**inputs)
    nc = _build_program()
    res = run_bass_kernel_spmd(nc, in_maps, core_ids=list(range(NCORE)),
                               **RUN_KW)
    LAST["res"] = res
    out = np.concatenate([r["out"].T for r in res.results], axis=0)
    return out.astype(np.float32)
